# revision 17
# baseline (speedup 1.0000x reference)
"""Trainium2 Bass kernel for nn_AttentionBlock (GroupNorm + single-head
self-attention over HW tokens + proj + residual).

Strategy: data-parallel over batch (B=32 -> 4 images per core on 8 cores),
all parameters replicated. All heavy matmuls run in float32r (full fp32
storage, ~bf16 PE throughput at N=512, ~1e-4 multiply precision).

Key algebraic folds (host-side, exact):
  - proj is folded into V: u := (proj_w @ W_v) h, so attn@V directly
    produces the projected output; the V bias folds into an effective
    output bias because softmax weights sum to 1.
  - the K bias is dropped entirely (softmax-invariant).
  - softmax normalization is deferred: O_unnorm accumulates in PSUM and is
    scaled by 1/rowsum at eviction; rowsums come from an all-ones matmul
    which lands the sums broadcast across all partitions.

Self-contained: hardcodes shapes from the problem spec; no sibling imports.
"""
import contextlib
import sys
import types

import numpy as np
import orjson

import concourse.bass as bass
import concourse.tile as tile
from concourse import mybir
from concourse import bass_utils

F32 = mybir.dt.float32
F32R = mybir.dt.float32r
AF = mybir.ActivationFunctionType
ALU = mybir.AluOpType
AX = mybir.AxisListType
ts = bass.ts

# ---------------------------------------------------------------------------
# Problem constants (hardcoded per spec)
B, C, H, W = 32, 512, 32, 32
HW = H * W                      # 1024 tokens per image
GROUPS = 8
GSIZE = C // GROUPS             # 64 channels per group
EPS = 1e-5
SCALE = C ** (-0.5)             # attention scale (N_HEADS=1)
NCORES = 8
BSH = B // NCORES               # images per core
CT = C // 128                   # 4 channel partition-tiles
MT = HW // 128                  # 8 token partition-tiles
NH = HW // 512                  # 2 free-dim halves of the token axis
GN_N = GSIZE * HW               # elements per group (65536)


# ---------------------------------------------------------------------------
# Workaround: this walrus build only accepts 1 sync-wait command per
# instruction; Tile's exit drain carries one wait per outstanding semaphore.
# Split excess waits onto preceding NoOps at the BIR JSON level.
def _split_waits_json(bir_bytes, max_waits=1):
    j = orjson.loads(bir_bytes)
    for func in j["functions"]:
        for bb in func["blocks"]:
            out = []
            for ins in bb["instructions"]:
                si = ins.get("sync_info")
                waits = si.get("on_wait") if si else None
                if waits and len(waits) > max_waits:
                    excess = waits[: len(waits) - max_waits]
                    ins["sync_info"]["on_wait"] = waits[len(waits) - max_waits:]
                    for i in range(0, len(excess), max_waits):
                        out.append({
                            "name": f"{ins['name']}__wsplit{i}",
                            "opcode": "NoOp",
                            "engine": ins["engine"],
                            "ins": [],
                            "outs": [],
                            "sync_info": {"on_update": [],
                                          "on_wait": excess[i:i + max_waits]},
                        })
                out.append(ins)
            bb["instructions"] = out
    return orjson.dumps(j)


_ORIG_TO_JSON = bass.Bass.to_json_bytes
if getattr(bass.Bass, "_ant_wait_split", False) is False:
    bass.Bass.to_json_bytes = lambda self: _split_waits_json(_ORIG_TO_JSON(self))
    bass.Bass._ant_wait_split = True


# ---------------------------------------------------------------------------
# Optional: register the axon NTFF profile hook (image's antenv lacks it).
def install_trace_hook():
    if "antenv.axon_hooks" in sys.modules:
        return
    try:
        import antenv
        from trn_agent_boot.trn_boot import _ntff_profile_via_ctypes
    except Exception:
        return
    mod = types.ModuleType("antenv.axon_hooks")
    _state = {"hook": None}
    mod.set_axon_ntff_profile_hook = lambda h: _state.__setitem__("hook", h)
    mod.get_axon_ntff_profile_hook = lambda: _state["hook"]
    sys.modules["antenv.axon_hooks"] = mod
    antenv.axon_hooks = mod
    try:
        mod.set_axon_ntff_profile_hook(
            _ntff_profile_via_ctypes("/opt/axon/libaxon_pjrt.so"))
    except Exception:
        sys.modules.pop("antenv.axon_hooks", None)


# ---------------------------------------------------------------------------
class _Ctx:
    """Shared build context."""

    def __init__(self, nc, pools, consts, x_dram, y_dram):
        self.nc = nc
        self.pools = pools
        self.consts = consts
        self.x_dram = x_dram
        self.y_dram = y_dram


def _load_x(cx, img):
    nc = cx.nc
    xp = cx.pools["xp"]
    x_sb = xp.tile([128, CT, HW], F32, name=f"x_i{img}", tag="x", bufs=3)
    xr = cx.x_dram[img].rearrange("(t p) m -> p t m", p=128)
    for t in range(CT):
        nc.sync.dma_start(x_sb[:, t, :], xr[:, t, :])
    return x_sb


def _emit_gn_a(cx, img, x_sb):
    """GroupNorm part A: per-channel partial sums / sums of squares."""
    nc, co = cx.nc, cx.consts
    sb, ps = cx.pools["sb"], cx.pools["ps"]
    nm = f"i{img}"
    with nc.named_scope(f"gn{img}"):
        # per-channel partials: [:, 0, t] = sum over tokens, [:, 1, t] = sumsq
        part = sb.tile([128, 2, CT], F32, name=f"part_{nm}", tag="part")
        sq_scr = sb.tile([128, HW], F32, name=f"sqscr_{nm}", tag="sqscr")
        for t in range(CT):
            nc.vector.reduce_sum(part[:, 0, t:t + 1], x_sb[:, t, :], axis=AX.X)
            nc.vector.scalar_tensor_tensor(sq_scr[:], x_sb[:, t, :], 1.0,
                                           x_sb[:, t, :], op0=ALU.mult,
                                           op1=ALU.mult,
                                           accum_out=part[:, 1, t:t + 1])
    return {"x": x_sb, "part": part}


def _emit_gn_b(cx, img, gs):
    """GroupNorm part B: group stats, per-channel affine, apply -> h."""
    nc, co = cx.nc, cx.consts
    sb, ps = cx.pools["sb"], cx.pools["ps"]
    nm = f"i{img}"
    x_sb, part = gs["x"], gs["part"]
    with nc.named_scope(f"gn{img}"):
        # group totals via selector matmuls: psum_st[g, {sum,sumsq}]
        ps_st = ps.tile([GROUPS, 2], F32, name=f"ps_st_{nm}", tag="ps")
        for t in range(CT):
            nc.tensor.matmul(ps_st[:], co["sel"][:, t, :], part[:, :, t],
                             start=(t == 0), stop=(t == CT - 1))
        # stats: [g, 0] = mean, [g, 1] = E[x^2] -> rstd
        stats = sb.tile([GROUPS, 2], F32, name=f"stats_{nm}", tag="stats")
        nc.scalar.activation(stats[:], ps_st[:], AF.Copy, scale=1.0 / GN_N)
        var = sb.tile([GROUPS, 1], F32, name=f"var_{nm}", tag="var")
        nc.vector.tensor_mul(var[:], stats[:, 0:1], stats[:, 0:1])
        nc.vector.tensor_sub(var[:], stats[:, 1:2], var[:])
        nc.scalar.activation(var[:], var[:], AF.Sqrt, bias=co["eps"][0:GROUPS])
        nc.vector.reciprocal(stats[:, 1:2], var[:])

        # broadcast per-channel mean/rstd, fold gamma/beta into scale/shift
        scl = sb.tile([128, CT], F32, name=f"scl_{nm}", tag="scl")
        shf = sb.tile([128, CT], F32, name=f"shf_{nm}", tag="shf")
        nscl = sb.tile([128, CT], F32, name=f"nscl_{nm}", tag="nscl")
        h_sb = [sb.tile([128, HW], F32R, name=f"h{t}_{nm}", tag=f"h{t}",
                        bufs=2) for t in range(CT)]
        for t in range(CT):
            ps_bc = ps.tile([128, 2], F32, name=f"ps_bc{t}_{nm}", tag="ps")
            nc.tensor.matmul(ps_bc[:], co["bsel"][:, t, :], stats[:],
                             start=True, stop=True)
            # scale = rstd*gamma ; shift = beta - mean*scale (via neg-gamma)
            nc.vector.tensor_mul(scl[:, t:t + 1], ps_bc[:, 1:2],
                                 co["gma"][:, t:t + 1])
            nc.vector.tensor_mul(nscl[:, t:t + 1], ps_bc[:, 1:2],
                                 co["ngma"][:, t:t + 1])
            nc.vector.scalar_tensor_tensor(shf[:, t:t + 1], ps_bc[:, 0:1],
                                           nscl[:, t:t + 1],
                                           co["bta"][:, t:t + 1],
                                           op0=ALU.mult, op1=ALU.add)
            # h = x*scale + shift  (rounded to f32r, on ACT: DVE is busier)
            nc.scalar.activation(h_sb[t][:], x_sb[:, t, :], AF.Identity,
                                 bias=shf[:, t:t + 1], scale=scl[:, t:t + 1])
    gs["h"] = h_sb
    return gs


def _emit_front(cx, img, gs):
    """q, k, u projections and exp(scaled S^T) for one image."""
    nc, co = cx.nc, cx.consts
    sb, ps = cx.pools["sb"], cx.pools["ps"]
    nm = f"i{img}"
    h_sb = gs["h"]

    # ---- q, k: [c_out partition-tiles, HW] = wqkT.T @ h ----
    with nc.named_scope(f"qk{img}"):
        qk_sb = []
        for j in range(2 * CT):          # 0..3 = q tiles, 4..7 = k tiles
            q_t = sb.tile([128, HW], F32R, name=f"qk{j}_{nm}", tag=f"qk{j}")
            for h_ in range(NH):
                p = ps.tile([128, 512], F32, name=f"ps_qk{j}h{h_}_{nm}",
                            tag="ps")
                for kt in range(CT):
                    nc.tensor.matmul(p[:], co["wqkT"][kt][:, ts(j, 128)],
                                     h_sb[kt][:, ts(h_, 512)],
                                     start=(kt == 0), stop=(kt == CT - 1))
                if j < CT:
                    # q bias; the k bias is softmax-invariant and dropped
                    nc.vector.tensor_scalar_add(q_t[:, ts(h_, 512)], p[:],
                                                co["qkb"][:, j:j + 1])
                else:
                    nc.scalar.copy(q_t[:, ts(h_, 512)], p[:])
            qk_sb.append(q_t)
        q_sb, k_sb = qk_sb[:CT], qk_sb[CT:]

    # ---- u token-major: [m partition-tiles, C] = h.T @ (proj@Wv).T ----
    with nc.named_scope(f"u{img}"):
        u_sb = []
        for mt in range(MT):
            u_t = sb.tile([128, C], F32R, name=f"u{mt}_{nm}", tag=f"u{mt}")
            p = ps.tile([128, 512], F32, name=f"ps_u{mt}_{nm}", tag="ps")
            for kt in range(CT):
                nc.tensor.matmul(p[:], h_sb[kt][:, ts(mt, 128)],
                                 co["wpvT"][kt][:],
                                 start=(kt == 0), stop=(kt == CT - 1))
            nc.scalar.copy(u_t[:], p[:])
            u_sb.append(u_t)

    return {"u": u_sb, "q": q_sb, "k": k_sb}


def _emit_st(cx, img, fs):
    """S^T and exp: at[mt][:, n] = exp(SCALE * sum_c k[c,m] q[c,n])."""
    nc = cx.nc
    sb, ps = cx.pools["sb"], cx.pools["ps"]
    nm = f"i{img}"
    q_sb, k_sb = fs["q"], fs["k"]
    with nc.named_scope(f"st{img}"):
        at_sb = [sb.tile([128, HW], F32R, name=f"at{mt}_{nm}", tag=f"at{mt}")
                 for mt in range(MT)]
        for mt in range(MT):
            for h_ in range(NH):
                p = ps.tile([128, 512], F32, name=f"ps_s{mt}h{h_}_{nm}",
                            tag="ps")
                for kt in range(CT):
                    nc.tensor.matmul(p[:], k_sb[kt][:, ts(mt, 128)],
                                     q_sb[kt][:, ts(h_, 512)],
                                     start=(kt == 0), stop=(kt == CT - 1))
                nc.scalar.activation(at_sb[mt][:, ts(h_, 512)], p[:], AF.Exp,
                                     scale=SCALE)
    fs["at"] = at_sb
    return fs


def _emit_back(cx, img, gs, fs, h_):
    """Row sums, attn @ u accumulation, normalize + bias + residual, store."""
    nc, co = cx.nc, cx.consts
    sb, ps, yp = cx.pools["sb"], cx.pools["ps"], cx.pools["yp"]
    nm = f"i{img}"
    x_sb, u_sb, at_sb = gs["x"], fs["u"], fs["at"]

    if h_ == 0:
        fs["invrs"] = sb.tile([128, HW], F32, name=f"invrs_{nm}", tag="invrs")
    invrs = fs["invrs"]
    with nc.named_scope(f"y{img}"):
        if True:  # single half h_
            # all-ones lhsT puts sum_m at[m, n] on every partition
            ps_rs = ps.tile([128, 512], F32, name=f"ps_rs{h_}_{nm}", tag="ps")
            for mt in range(MT):
                nc.tensor.matmul(ps_rs[:], co["ones"][:],
                                 at_sb[mt][:, ts(h_, 512)],
                                 start=(mt == 0), stop=(mt == MT - 1))
            nc.vector.reciprocal(invrs[:, ts(h_, 512)], ps_rs[:])
            ps_ot = [ps.tile([128, 512], F32, name=f"ps_ot{ct}h{h_}_{nm}",
                             tag="ps") for ct in range(CT)]
            for mt in range(MT):
                for ct in range(CT):
                    nc.tensor.matmul(ps_ot[ct][:], u_sb[mt][:, ts(ct, 128)],
                                     at_sb[mt][:, ts(h_, 512)],
                                     start=(mt == 0), stop=(mt == MT - 1))
            for ct in range(CT):
                tmp = sb.tile([128, 512], F32, name=f"tmp{ct}h{h_}_{nm}",
                              tag="tmp", bufs=1)
                nc.vector.tensor_mul(tmp[:], ps_ot[ct][:],
                                     invrs[:, ts(h_, 512)])
                y_t = yp.tile([128, 512], F32, name=f"y{ct}h{h_}_{nm}",
                              tag="y", bufs=4)
                nc.vector.scalar_tensor_tensor(
                    y_t[:], tmp[:], co["pjb"][:, ct:ct + 1],
                    x_sb[:, ct, ts(h_, 512)], op0=ALU.add, op1=ALU.add)
                nc.sync.dma_start(
                    cx.y_dram[img, ts(ct, 128), bass.ds(h_ * 512, 512)],
                    y_t[:])


def build(n_img=BSH):
    nc = bass.Bass(trn_type="TRN2", target_bir_lowering=False, debug=False)
    x_dram = nc.dram_tensor("x", [n_img, C, HW], F32, kind="ExternalInput").ap()
    wqk_dram = nc.dram_tensor("wqkT", [C, 2 * C], F32R,
                              kind="ExternalInput").ap()
    wpv_dram = nc.dram_tensor("wpvT", [C, C], F32R, kind="ExternalInput").ap()
    qkb_dram = nc.dram_tensor("qkb", [128, CT], F32, kind="ExternalInput").ap()
    pjb_dram = nc.dram_tensor("pjb", [128, CT], F32, kind="ExternalInput").ap()
    gma_dram = nc.dram_tensor("gma", [128, CT], F32, kind="ExternalInput").ap()
    ngma_dram = nc.dram_tensor("ngma", [128, CT], F32,
                               kind="ExternalInput").ap()
    bta_dram = nc.dram_tensor("bta", [128, CT], F32, kind="ExternalInput").ap()
    sel_dram = nc.dram_tensor("sel", [128, CT, GROUPS], F32,
                              kind="ExternalInput").ap()
    bsel_dram = nc.dram_tensor("bsel", [GROUPS, CT, 128], F32,
                               kind="ExternalInput").ap()
    ones_dram = nc.dram_tensor("ones", [128, 128], F32R,
                               kind="ExternalInput").ap()
    y_dram = nc.dram_tensor("y", [n_img, C, HW], F32, kind="ExternalOutput").ap()

    with tile.TileContext(nc) as tc:
        with contextlib.ExitStack() as ctx:
            wp_pool = ctx.enter_context(tc.tile_pool(name="wp", bufs=1))
            sb = ctx.enter_context(tc.tile_pool(name="sb", bufs=1))
            xp = ctx.enter_context(tc.tile_pool(name="xp", bufs=2))
            yp = ctx.enter_context(tc.tile_pool(name="yp", bufs=3))
            ps = ctx.enter_context(tc.tile_pool(name="ps", bufs=8,
                                                space="PSUM"))

            def load(dram_ap, shape, name, dt=F32):
                t = wp_pool.tile(shape, dt, name=name, tag=name)
                nc.sync.dma_start(t[:], dram_ap)
                return t

            wqk_r = wqk_dram.rearrange("(t p) o -> p t o", p=128)
            wpv_r = wpv_dram.rearrange("(t p) o -> p t o", p=128)
            consts = {
                "wqkT": [load(wqk_r[:, t, :], [128, 2 * C], f"wqkT{t}", F32R)
                         for t in range(CT)],
                "wpvT": [load(wpv_r[:, t, :], [128, C], f"wpvT{t}", F32R)
                         for t in range(CT)],
                "qkb": load(qkb_dram, [128, CT], "qkb"),
                "pjb": load(pjb_dram, [128, CT], "pjb"),
                "gma": load(gma_dram, [128, CT], "gma"),
                "ngma": load(ngma_dram, [128, CT], "ngma"),
                "bta": load(bta_dram, [128, CT], "bta"),
                "sel": load(sel_dram, [128, CT, GROUPS], "sel"),
                "bsel": load(bsel_dram, [GROUPS, CT, 128], "bsel"),
                "ones": load(ones_dram, [128, 128], "ones", F32R),
            }
            eps_t = wp_pool.tile([128, 1], F32, name="eps", tag="eps")
            nc.vector.memset(eps_t[:], EPS)
            consts["eps"] = eps_t

            cx = _Ctx(nc, dict(sb=sb, ps=ps, xp=xp, yp=yp), consts,
                      x_dram, y_dram)

            # software pipeline: x loads run two images ahead; groupnorm of
            # image i+1 is emitted in two parts inside image i so the PE
            # never idles (and HAM stays at full clock) across images.
            # PE warmup: ~4us of matmuls so HAM unthrottles before real work
            wa = wp_pool.tile([128, 128], F32, name="warm", tag="warm")
            nc.vector.memset(wa[:], 1.0)
            for i in range(10):
                pw = ps.tile([128, 128], F32, name=f"pw{i}", tag="ps")
                nc.tensor.matmul(pw[:], wa[:], wa[:], start=True, stop=True)

            xs = [_load_x(cx, i) for i in range(min(2, n_img))]
            gs = [_emit_gn_b(cx, 0, _emit_gn_a(cx, 0, xs[0]))]
            for img in range(n_img):
                fs = _emit_front(cx, img, gs[img])
                if img + 2 < n_img:
                    xs.append(_load_x(cx, img + 2))
                if img + 1 < n_img:
                    gs.append(_emit_gn_a(cx, img + 1, xs[img + 1]))
                _emit_st(cx, img, fs)
                _emit_back(cx, img, gs[img], fs, 0)
                if img + 1 < n_img:
                    _emit_gn_b(cx, img + 1, gs[img + 1])
                _emit_back(cx, img, gs[img], fs, 1)
    return nc


# ---------------------------------------------------------------------------
def _host_inputs(x, norm_w, norm_b, qkv_w, qkv_b, proj_w, proj_b, n_img):
    """Build per-core input maps (host-side layout prep + weight folds)."""
    x = np.ascontiguousarray(np.asarray(x, dtype=np.float32).reshape(B, C, HW))
    qkv_w = np.asarray(qkv_w, dtype=np.float64)
    proj_w = np.asarray(proj_w, dtype=np.float64)
    w_pv = proj_w @ qkv_w[2 * C:]                     # [C, C] folded proj@Wv
    pjb_eff = (np.asarray(proj_b, np.float64)
               + proj_w @ np.asarray(qkv_b, np.float64)[2 * C:])
    com = {
        "wqkT": np.ascontiguousarray(qkv_w[:2 * C].T, dtype=np.float32),
        "wpvT": np.ascontiguousarray(w_pv.T, dtype=np.float32),
        "qkb": np.ascontiguousarray(
            np.asarray(qkv_b, np.float32)[:C].reshape(CT, 128).T),
        "pjb": np.ascontiguousarray(
            pjb_eff.astype(np.float32).reshape(CT, 128).T),
        "gma": np.ascontiguousarray(
            np.asarray(norm_w, np.float32).reshape(CT, 128).T),
        "ngma": np.ascontiguousarray(
            -np.asarray(norm_w, np.float32).reshape(CT, 128).T),
        "bta": np.ascontiguousarray(
            np.asarray(norm_b, np.float32).reshape(CT, 128).T),
        "ones": np.ones((128, 128), np.float32),
    }
    sel = np.zeros((128, CT, GROUPS), np.float32)
    bsel = np.zeros((GROUPS, CT, 128), np.float32)
    for t in range(CT):
        for p in range(128):
            g = (t * 128 + p) // GSIZE
            sel[p, t, g] = 1.0
            bsel[g, t, p] = 1.0
    com["sel"] = sel
    com["bsel"] = bsel

    in_maps = []
    for i in range(NCORES):
        m = dict(com)
        m["x"] = np.ascontiguousarray(x[i * n_img:(i + 1) * n_img])
        in_maps.append(m)
    return in_maps


_NC_CACHE = {}


def run(inputs, trace=False, n_img=BSH, n_cores=NCORES):
    if trace:
        install_trace_hook()
    key = n_img
    if key not in _NC_CACHE:
        _NC_CACHE[key] = build(n_img)
    nc = _NC_CACHE[key]
    in_maps = _host_inputs(n_img=n_img, **inputs)[:n_cores]
    res = bass_utils.run_bass_kernel_spmd(
        nc, in_maps, core_ids=list(range(n_cores)), trace=trace)
    y = np.concatenate([r["y"] for r in res.results], axis=0)
    return y.reshape(n_cores * n_img, C, H, W), res


def kernel(**inputs):
    y, _ = run(inputs)
    return y.astype(np.float32)


# revision 18
# speedup vs baseline: 1.0099x; 1.0099x over previous
"""Trainium2 Bass kernel for nn_AttentionBlock (GroupNorm + single-head
self-attention over HW tokens + proj + residual).

Strategy: data-parallel over batch (B=32 -> 4 images per core on 8 cores),
all parameters replicated. All heavy matmuls run in float32r (full fp32
storage, ~bf16 PE throughput at N=512, ~1e-4 multiply precision).

Key algebraic folds (host-side, exact):
  - proj is folded into V: u := (proj_w @ W_v) h, so attn@V directly
    produces the projected output; the V bias folds into an effective
    output bias because softmax weights sum to 1.
  - the K bias is dropped entirely (softmax-invariant).
  - softmax normalization is deferred: O_unnorm accumulates in PSUM and is
    scaled by 1/rowsum at eviction; rowsums come from an all-ones matmul
    which lands the sums broadcast across all partitions.

Self-contained: hardcodes shapes from the problem spec; no sibling imports.
"""
import contextlib
import sys
import types

import numpy as np
import orjson

import concourse.bass as bass
import concourse.tile as tile
from concourse import mybir
from concourse import bass_utils

F32 = mybir.dt.float32
F32R = mybir.dt.float32r
AF = mybir.ActivationFunctionType
ALU = mybir.AluOpType
AX = mybir.AxisListType
ts = bass.ts

# ---------------------------------------------------------------------------
# Problem constants (hardcoded per spec)
B, C, H, W = 32, 512, 32, 32
HW = H * W                      # 1024 tokens per image
GROUPS = 8
GSIZE = C // GROUPS             # 64 channels per group
EPS = 1e-5
SCALE = C ** (-0.5)             # attention scale (N_HEADS=1)
NCORES = 8
BSH = B // NCORES               # images per core
CT = C // 128                   # 4 channel partition-tiles
MT = HW // 128                  # 8 token partition-tiles
NH = HW // 512                  # 2 free-dim halves of the token axis
GN_N = GSIZE * HW               # elements per group (65536)


# ---------------------------------------------------------------------------
# Workaround: this walrus build only accepts 1 sync-wait command per
# instruction; Tile's exit drain carries one wait per outstanding semaphore.
# Split excess waits onto preceding NoOps at the BIR JSON level.
def _split_waits_json(bir_bytes, max_waits=1):
    j = orjson.loads(bir_bytes)
    for func in j["functions"]:
        for bb in func["blocks"]:
            out = []
            for ins in bb["instructions"]:
                si = ins.get("sync_info")
                waits = si.get("on_wait") if si else None
                if waits and len(waits) > max_waits:
                    excess = waits[: len(waits) - max_waits]
                    ins["sync_info"]["on_wait"] = waits[len(waits) - max_waits:]
                    for i in range(0, len(excess), max_waits):
                        out.append({
                            "name": f"{ins['name']}__wsplit{i}",
                            "opcode": "NoOp",
                            "engine": ins["engine"],
                            "ins": [],
                            "outs": [],
                            "sync_info": {"on_update": [],
                                          "on_wait": excess[i:i + max_waits]},
                        })
                out.append(ins)
            bb["instructions"] = out
    return orjson.dumps(j)


_ORIG_TO_JSON = bass.Bass.to_json_bytes
if getattr(bass.Bass, "_ant_wait_split", False) is False:
    bass.Bass.to_json_bytes = lambda self: _split_waits_json(_ORIG_TO_JSON(self))
    bass.Bass._ant_wait_split = True


# ---------------------------------------------------------------------------
# Optional: register the axon NTFF profile hook (image's antenv lacks it).
def install_trace_hook():
    if "antenv.axon_hooks" in sys.modules:
        return
    try:
        import antenv
        from trn_agent_boot.trn_boot import _ntff_profile_via_ctypes
    except Exception:
        return
    mod = types.ModuleType("antenv.axon_hooks")
    _state = {"hook": None}
    mod.set_axon_ntff_profile_hook = lambda h: _state.__setitem__("hook", h)
    mod.get_axon_ntff_profile_hook = lambda: _state["hook"]
    sys.modules["antenv.axon_hooks"] = mod
    antenv.axon_hooks = mod
    try:
        mod.set_axon_ntff_profile_hook(
            _ntff_profile_via_ctypes("/opt/axon/libaxon_pjrt.so"))
    except Exception:
        sys.modules.pop("antenv.axon_hooks", None)


# ---------------------------------------------------------------------------
class _Ctx:
    """Shared build context."""

    def __init__(self, nc, pools, consts, x_dram, y_dram):
        self.nc = nc
        self.pools = pools
        self.consts = consts
        self.x_dram = x_dram
        self.y_dram = y_dram


def _load_x(cx, img):
    nc = cx.nc
    xp = cx.pools["xp"]
    x_sb = xp.tile([128, CT, HW], F32, name=f"x_i{img}", tag="x", bufs=3)
    xr = cx.x_dram[img].rearrange("(t p) m -> p t m", p=128)
    for t in range(CT):
        nc.sync.dma_start(x_sb[:, t, :], xr[:, t, :])
    return x_sb


def _emit_gn_a(cx, img, x_sb):
    """GroupNorm part A: per-channel partial sums / sums of squares."""
    nc, co = cx.nc, cx.consts
    sb, ps = cx.pools["sb"], cx.pools["ps"]
    nm = f"i{img}"
    with nc.named_scope(f"gn{img}"):
        # per-channel partials: [:, 0, t] = sum over tokens, [:, 1, t] = sumsq
        part = sb.tile([128, 2, CT], F32, name=f"part_{nm}", tag="part")
        sq_scr = sb.tile([128, HW], F32, name=f"sqscr_{nm}", tag="sqscr")
        for t in range(CT):
            nc.vector.reduce_sum(part[:, 0, t:t + 1], x_sb[:, t, :], axis=AX.X)
            nc.vector.scalar_tensor_tensor(sq_scr[:], x_sb[:, t, :], 1.0,
                                           x_sb[:, t, :], op0=ALU.mult,
                                           op1=ALU.mult,
                                           accum_out=part[:, 1, t:t + 1])
    return {"x": x_sb, "part": part}


def _emit_gn_b(cx, img, gs):
    """GroupNorm part B: group stats, per-channel affine, apply -> h."""
    nc, co = cx.nc, cx.consts
    sb, ps = cx.pools["sb"], cx.pools["ps"]
    nm = f"i{img}"
    x_sb, part = gs["x"], gs["part"]
    with nc.named_scope(f"gn{img}"):
        # group totals via selector matmuls: psum_st[g, {sum,sumsq}]
        ps_st = ps.tile([GROUPS, 2], F32, name=f"ps_st_{nm}", tag="ps")
        for t in range(CT):
            nc.tensor.matmul(ps_st[:], co["sel"][:, t, :], part[:, :, t],
                             start=(t == 0), stop=(t == CT - 1))
        # stats: [g, 0] = mean, [g, 1] = E[x^2] -> rstd
        stats = sb.tile([GROUPS, 2], F32, name=f"stats_{nm}", tag="stats")
        nc.scalar.activation(stats[:], ps_st[:], AF.Copy, scale=1.0 / GN_N)
        var = sb.tile([GROUPS, 1], F32, name=f"var_{nm}", tag="var")
        nc.vector.tensor_mul(var[:], stats[:, 0:1], stats[:, 0:1])
        nc.vector.tensor_sub(var[:], stats[:, 1:2], var[:])
        nc.scalar.activation(var[:], var[:], AF.Sqrt, bias=co["eps"][0:GROUPS])
        nc.vector.reciprocal(stats[:, 1:2], var[:])

        # broadcast per-channel mean/rstd, fold gamma/beta into scale/shift
        scl = sb.tile([128, CT], F32, name=f"scl_{nm}", tag="scl")
        shf = sb.tile([128, CT], F32, name=f"shf_{nm}", tag="shf")
        nscl = sb.tile([128, CT], F32, name=f"nscl_{nm}", tag="nscl")
        h_sb = [sb.tile([128, HW], F32R, name=f"h{t}_{nm}", tag=f"h{t}",
                        bufs=2) for t in range(CT)]
        for t in range(CT):
            ps_bc = ps.tile([128, 2], F32, name=f"ps_bc{t}_{nm}", tag="ps")
            nc.tensor.matmul(ps_bc[:], co["bsel"][:, t, :], stats[:],
                             start=True, stop=True)
            # scale = rstd*gamma ; shift = beta - mean*scale (via neg-gamma)
            nc.vector.tensor_mul(scl[:, t:t + 1], ps_bc[:, 1:2],
                                 co["gma"][:, t:t + 1])
            nc.vector.tensor_mul(nscl[:, t:t + 1], ps_bc[:, 1:2],
                                 co["ngma"][:, t:t + 1])
            nc.vector.scalar_tensor_tensor(shf[:, t:t + 1], ps_bc[:, 0:1],
                                           nscl[:, t:t + 1],
                                           co["bta"][:, t:t + 1],
                                           op0=ALU.mult, op1=ALU.add)
            # h = x*scale + shift  (rounded to f32r, on ACT: DVE is busier)
            nc.scalar.activation(h_sb[t][:], x_sb[:, t, :], AF.Identity,
                                 bias=shf[:, t:t + 1], scale=scl[:, t:t + 1])
    gs["h"] = h_sb
    return gs


def _emit_front(cx, img, gs):
    """q, k, u projections and exp(scaled S^T) for one image."""
    nc, co = cx.nc, cx.consts
    sb, ps = cx.pools["sb"], cx.pools["ps"]
    nm = f"i{img}"
    h_sb = gs["h"]

    # ---- q, k: [c_out partition-tiles, HW] = wqkT.T @ h ----
    with nc.named_scope(f"qk{img}"):
        qk_sb = []
        for j in range(2 * CT):          # 0..3 = q tiles, 4..7 = k tiles
            q_t = sb.tile([128, HW], F32R, name=f"qk{j}_{nm}", tag=f"qk{j}")
            for h_ in range(NH):
                p = ps.tile([128, 512], F32, name=f"ps_qk{j}h{h_}_{nm}",
                            tag="ps")
                for kt in range(CT):
                    nc.tensor.matmul(p[:], co["wqkT"][kt][:, ts(j, 128)],
                                     h_sb[kt][:, ts(h_, 512)],
                                     start=(kt == 0), stop=(kt == CT - 1))
                if j < CT:
                    # q bias; the k bias is softmax-invariant and dropped
                    nc.vector.tensor_scalar_add(q_t[:, ts(h_, 512)], p[:],
                                                co["qkb"][:, j:j + 1])
                else:
                    nc.vector.tensor_copy(q_t[:, ts(h_, 512)], p[:])
            qk_sb.append(q_t)
        q_sb, k_sb = qk_sb[:CT], qk_sb[CT:]

    # ---- u token-major: [m partition-tiles, C] = h.T @ (proj@Wv).T ----
    with nc.named_scope(f"u{img}"):
        u_sb = []
        for mt in range(MT):
            u_t = sb.tile([128, C], F32R, name=f"u{mt}_{nm}", tag=f"u{mt}")
            p = ps.tile([128, 512], F32, name=f"ps_u{mt}_{nm}", tag="ps")
            for kt in range(CT):
                nc.tensor.matmul(p[:], h_sb[kt][:, ts(mt, 128)],
                                 co["wpvT"][kt][:],
                                 start=(kt == 0), stop=(kt == CT - 1))
            nc.scalar.copy(u_t[:], p[:])
            u_sb.append(u_t)

    return {"u": u_sb, "q": q_sb, "k": k_sb}


def _emit_st(cx, img, fs):
    """S^T and exp: at[mt][:, n] = exp(SCALE * sum_c k[c,m] q[c,n])."""
    nc = cx.nc
    sb, ps = cx.pools["sb"], cx.pools["ps"]
    nm = f"i{img}"
    q_sb, k_sb = fs["q"], fs["k"]
    with nc.named_scope(f"st{img}"):
        at_sb = [sb.tile([128, HW], F32R, name=f"at{mt}_{nm}", tag=f"at{mt}")
                 for mt in range(MT)]
        for mt in range(MT):
            for h_ in range(NH):
                p = ps.tile([128, 512], F32, name=f"ps_s{mt}h{h_}_{nm}",
                            tag="ps")
                for kt in range(CT):
                    nc.tensor.matmul(p[:], k_sb[kt][:, ts(mt, 128)],
                                     q_sb[kt][:, ts(h_, 512)],
                                     start=(kt == 0), stop=(kt == CT - 1))
                nc.scalar.activation(at_sb[mt][:, ts(h_, 512)], p[:], AF.Exp,
                                     scale=SCALE)
    fs["at"] = at_sb
    return fs


def _emit_back(cx, img, gs, fs, h_):
    """Row sums, attn @ u accumulation, normalize + bias + residual, store."""
    nc, co = cx.nc, cx.consts
    sb, ps, yp = cx.pools["sb"], cx.pools["ps"], cx.pools["yp"]
    nm = f"i{img}"
    x_sb, u_sb, at_sb = gs["x"], fs["u"], fs["at"]

    if h_ == 0:
        fs["invrs"] = sb.tile([128, HW], F32, name=f"invrs_{nm}", tag="invrs")
    invrs = fs["invrs"]
    with nc.named_scope(f"y{img}"):
        if True:  # single half h_
            # all-ones lhsT puts sum_m at[m, n] on every partition
            ps_rs = ps.tile([128, 512], F32, name=f"ps_rs{h_}_{nm}", tag="ps")
            for mt in range(MT):
                nc.tensor.matmul(ps_rs[:], co["ones"][:],
                                 at_sb[mt][:, ts(h_, 512)],
                                 start=(mt == 0), stop=(mt == MT - 1))
            nc.vector.reciprocal(invrs[:, ts(h_, 512)], ps_rs[:])
            ps_ot = [ps.tile([128, 512], F32, name=f"ps_ot{ct}h{h_}_{nm}",
                             tag="ps") for ct in range(CT)]
            for mt in range(MT):
                for ct in range(CT):
                    nc.tensor.matmul(ps_ot[ct][:], u_sb[mt][:, ts(ct, 128)],
                                     at_sb[mt][:, ts(h_, 512)],
                                     start=(mt == 0), stop=(mt == MT - 1))
            for ct in range(CT):
                tmp = sb.tile([128, 512], F32, name=f"tmp{ct}h{h_}_{nm}",
                              tag="tmp", bufs=1)
                nc.vector.tensor_mul(tmp[:], ps_ot[ct][:],
                                     invrs[:, ts(h_, 512)])
                y_t = yp.tile([128, 512], F32, name=f"y{ct}h{h_}_{nm}",
                              tag="y", bufs=4)
                nc.vector.scalar_tensor_tensor(
                    y_t[:], tmp[:], co["pjb"][:, ct:ct + 1],
                    x_sb[:, ct, ts(h_, 512)], op0=ALU.add, op1=ALU.add)
                nc.sync.dma_start(
                    cx.y_dram[img, ts(ct, 128), bass.ds(h_ * 512, 512)],
                    y_t[:])


def build(n_img=BSH):
    nc = bass.Bass(trn_type="TRN2", target_bir_lowering=False, debug=False)
    x_dram = nc.dram_tensor("x", [n_img, C, HW], F32, kind="ExternalInput").ap()
    wqk_dram = nc.dram_tensor("wqkT", [C, 2 * C], F32R,
                              kind="ExternalInput").ap()
    wpv_dram = nc.dram_tensor("wpvT", [C, C], F32R, kind="ExternalInput").ap()
    qkb_dram = nc.dram_tensor("qkb", [128, CT], F32, kind="ExternalInput").ap()
    pjb_dram = nc.dram_tensor("pjb", [128, CT], F32, kind="ExternalInput").ap()
    gma_dram = nc.dram_tensor("gma", [128, CT], F32, kind="ExternalInput").ap()
    ngma_dram = nc.dram_tensor("ngma", [128, CT], F32,
                               kind="ExternalInput").ap()
    bta_dram = nc.dram_tensor("bta", [128, CT], F32, kind="ExternalInput").ap()
    sel_dram = nc.dram_tensor("sel", [128, CT, GROUPS], F32,
                              kind="ExternalInput").ap()
    bsel_dram = nc.dram_tensor("bsel", [GROUPS, CT, 128], F32,
                               kind="ExternalInput").ap()
    ones_dram = nc.dram_tensor("ones", [128, 128], F32R,
                               kind="ExternalInput").ap()
    y_dram = nc.dram_tensor("y", [n_img, C, HW], F32, kind="ExternalOutput").ap()

    with tile.TileContext(nc) as tc:
        with contextlib.ExitStack() as ctx:
            wp_pool = ctx.enter_context(tc.tile_pool(name="wp", bufs=1))
            sb = ctx.enter_context(tc.tile_pool(name="sb", bufs=1))
            xp = ctx.enter_context(tc.tile_pool(name="xp", bufs=2))
            yp = ctx.enter_context(tc.tile_pool(name="yp", bufs=3))
            ps = ctx.enter_context(tc.tile_pool(name="ps", bufs=8,
                                                space="PSUM"))

            def load(dram_ap, shape, name, dt=F32):
                t = wp_pool.tile(shape, dt, name=name, tag=name)
                nc.sync.dma_start(t[:], dram_ap)
                return t

            wqk_r = wqk_dram.rearrange("(t p) o -> p t o", p=128)
            wpv_r = wpv_dram.rearrange("(t p) o -> p t o", p=128)
            consts = {
                "wqkT": [load(wqk_r[:, t, :], [128, 2 * C], f"wqkT{t}", F32R)
                         for t in range(CT)],
                "wpvT": [load(wpv_r[:, t, :], [128, C], f"wpvT{t}", F32R)
                         for t in range(CT)],
                "qkb": load(qkb_dram, [128, CT], "qkb"),
                "pjb": load(pjb_dram, [128, CT], "pjb"),
                "gma": load(gma_dram, [128, CT], "gma"),
                "ngma": load(ngma_dram, [128, CT], "ngma"),
                "bta": load(bta_dram, [128, CT], "bta"),
                "sel": load(sel_dram, [128, CT, GROUPS], "sel"),
                "bsel": load(bsel_dram, [GROUPS, CT, 128], "bsel"),
                "ones": load(ones_dram, [128, 128], "ones", F32R),
            }
            eps_t = wp_pool.tile([128, 1], F32, name="eps", tag="eps")
            nc.vector.memset(eps_t[:], EPS)
            consts["eps"] = eps_t

            cx = _Ctx(nc, dict(sb=sb, ps=ps, xp=xp, yp=yp), consts,
                      x_dram, y_dram)

            # software pipeline: x loads run two images ahead; groupnorm of
            # image i+1 is emitted in two parts inside image i so the PE
            # never idles (and HAM stays at full clock) across images.
            # PE warmup: ~4us of matmuls so HAM unthrottles before real work
            wa = wp_pool.tile([128, 128], F32, name="warm", tag="warm")
            nc.vector.memset(wa[:], 1.0)
            for i in range(10):
                pw = ps.tile([128, 128], F32, name=f"pw{i}", tag="ps")
                nc.tensor.matmul(pw[:], wa[:], wa[:], start=True, stop=True)

            xs = [_load_x(cx, i) for i in range(min(2, n_img))]
            gs = [_emit_gn_b(cx, 0, _emit_gn_a(cx, 0, xs[0]))]
            for img in range(n_img):
                fs = _emit_front(cx, img, gs[img])
                if img + 2 < n_img:
                    xs.append(_load_x(cx, img + 2))
                if img + 1 < n_img:
                    gs.append(_emit_gn_a(cx, img + 1, xs[img + 1]))
                _emit_st(cx, img, fs)
                if img + 1 < n_img:
                    _emit_gn_b(cx, img + 1, gs[img + 1])
                _emit_back(cx, img, gs[img], fs, 0)
                _emit_back(cx, img, gs[img], fs, 1)
    return nc


# ---------------------------------------------------------------------------
def _host_inputs(x, norm_w, norm_b, qkv_w, qkv_b, proj_w, proj_b, n_img):
    """Build per-core input maps (host-side layout prep + weight folds)."""
    x = np.ascontiguousarray(np.asarray(x, dtype=np.float32).reshape(B, C, HW))
    qkv_w = np.asarray(qkv_w, dtype=np.float64)
    proj_w = np.asarray(proj_w, dtype=np.float64)
    w_pv = proj_w @ qkv_w[2 * C:]                     # [C, C] folded proj@Wv
    pjb_eff = (np.asarray(proj_b, np.float64)
               + proj_w @ np.asarray(qkv_b, np.float64)[2 * C:])
    com = {
        "wqkT": np.ascontiguousarray(qkv_w[:2 * C].T, dtype=np.float32),
        "wpvT": np.ascontiguousarray(w_pv.T, dtype=np.float32),
        "qkb": np.ascontiguousarray(
            np.asarray(qkv_b, np.float32)[:C].reshape(CT, 128).T),
        "pjb": np.ascontiguousarray(
            pjb_eff.astype(np.float32).reshape(CT, 128).T),
        "gma": np.ascontiguousarray(
            np.asarray(norm_w, np.float32).reshape(CT, 128).T),
        "ngma": np.ascontiguousarray(
            -np.asarray(norm_w, np.float32).reshape(CT, 128).T),
        "bta": np.ascontiguousarray(
            np.asarray(norm_b, np.float32).reshape(CT, 128).T),
        "ones": np.ones((128, 128), np.float32),
    }
    sel = np.zeros((128, CT, GROUPS), np.float32)
    bsel = np.zeros((GROUPS, CT, 128), np.float32)
    for t in range(CT):
        for p in range(128):
            g = (t * 128 + p) // GSIZE
            sel[p, t, g] = 1.0
            bsel[g, t, p] = 1.0
    com["sel"] = sel
    com["bsel"] = bsel

    in_maps = []
    for i in range(NCORES):
        m = dict(com)
        m["x"] = np.ascontiguousarray(x[i * n_img:(i + 1) * n_img])
        in_maps.append(m)
    return in_maps


_NC_CACHE = {}


def run(inputs, trace=False, n_img=BSH, n_cores=NCORES):
    if trace:
        install_trace_hook()
    key = n_img
    if key not in _NC_CACHE:
        _NC_CACHE[key] = build(n_img)
    nc = _NC_CACHE[key]
    in_maps = _host_inputs(n_img=n_img, **inputs)[:n_cores]
    res = bass_utils.run_bass_kernel_spmd(
        nc, in_maps, core_ids=list(range(n_cores)), trace=trace)
    y = np.concatenate([r["y"] for r in res.results], axis=0)
    return y.reshape(n_cores * n_img, C, H, W), res


def kernel(**inputs):
    y, _ = run(inputs)
    return y.astype(np.float32)
